# revision 42
# baseline (speedup 1.0000x reference)
"""Block-causal attention (B=8, S=1024, D=1024, H=16, hd=64) on 8 TRN2 cores.

Sharding: data-parallel over batch — core b computes batch b end-to-end,
weights replicated, no collectives.

Per-core layout strategy (all host-side prep is free):
  - host passes x[b].T           -> xT   [D, S]
  - host passes de-interleaved   -> wqT, wkT  [D, D]  (RoPE pairs (2m,2m+1)
    permuted to (m, m+32) within each head's 64 rows, then transposed)
  - host passes wv.T, wo.T       -> wvT, woT  [D, D]
  - qT,kT computed in [D, S] layout (stationary = weight tile)
  - v computed in natural [S, D] layout (stationary = xT tile), stored with a
    ones-column per head (65 cols) so the attn@v matmul also produces the
    softmax normalizer Z as psum row 64
  - scores computed transposed sT[k, q] per (head, k-tile); softmax over the
    partition dim k is folded into the v-matmul via the ones column
  - final out[s, j] computed naturally (stationary = attn-out tile), divided
    attn-out by Z beforehand via partition-broadcast multiply
"""

import sys

sys.path.insert(0, "/opt/trn_rl_repo")

import numpy as np

import concourse.bass as bass  # noqa: F401
import concourse.mybir as mybir
import concourse.tile as tile
from concourse import bacc
from concourse.bass_utils import run_bass_kernel_spmd

B, S, D, H, HD = 8, 1024, 1024, 16, 64
P = 128          # partitions / tile
NT = D // P      # 8 tiles along D or S
BLK = 8          # mask block size
N_CORES = 8
F32 = mybir.dt.float32

BF16 = mybir.dt.bfloat16


def _build():
    nc = bacc.Bacc(
        "TRN2", target_bir_lowering=False, debug=False, num_devices=N_CORES
    )
    xT = nc.dram_tensor("xT", [D, S], BF16, kind="ExternalInput").ap()
    wqT = nc.dram_tensor("wqT", [D, D], BF16, kind="ExternalInput").ap()
    wkT = nc.dram_tensor("wkT", [D, D], BF16, kind="ExternalInput").ap()
    wvT = nc.dram_tensor("wvT", [D, D], BF16, kind="ExternalInput").ap()
    woT = nc.dram_tensor("woT", [D, D], BF16, kind="ExternalInput").ap()
    cosx = nc.dram_tensor("cosx", [P, S], BF16, kind="ExternalInput").ap()
    sinx = nc.dram_tensor("sinx", [P, S], BF16, kind="ExternalInput").ap()
    maskm = nc.dram_tensor("maskm", [P, P], BF16, kind="ExternalInput").ap()
    sel2d = nc.dram_tensor("sel2", [2, P], BF16, kind="ExternalInput").ap()
    out = nc.dram_tensor("out", [S, D], F32, kind="ExternalOutput").ap()

    ACF = mybir.ActivationFunctionType

    with tile.TileContext(nc) as tc:
        with (
            tc.tile_pool(name="big", bufs=8) as bigp,      # xT tiles (bf16)
            tc.tile_pool(name="aop", bufs=8) as aop,       # attn-out tiles
            tc.tile_pool(name="rot", bufs=10) as rotp,      # qT_rot + kT_rot stream
            tc.tile_pool(name="v65", bufs=8) as vp,        # v with ones cols
            tc.tile_pool(name="wt", bufs=4) as wtp,        # q/k weight m-blocks
            tc.tile_pool(name="wtv", bufs=16) as wtvp,     # v/wo weight chunks
            tc.tile_pool(name="tmp", bufs=6) as tmpp,      # plain + swapped
            tc.tile_pool(name="ex", bufs=8) as expp,       # exp(scores) tiles
            tc.tile_pool(name="const", bufs=1) as cp,
            tc.tile_pool(name="ob", bufs=2) as obp,        # output staging
            tc.tile_pool(name="st", bufs=3) as stp,        # psum->sbuf stage
            tc.tile_pool(name="psA", bufs=2, space="PSUM") as psA,  # 2 banks
            tc.tile_pool(name="psS", bufs=2, space="PSUM") as psS,  # 4 banks
            tc.tile_pool(name="psO", bufs=2, space="PSUM") as psO,  # 2 banks
        ):
            # ---- constants ----
            cos_t = cp.tile([P, S], BF16, tag="cos")
            sin_t = cp.tile([P, S], BF16, tag="sin")
            mask_t = cp.tile([P, P], BF16, tag="mask")
            zpf = {}  # per-pair [2, S] f32 Z tiles
            sel2 = cp.tile([2, P], BF16, tag="sel2")
            ones_f32 = cp.tile([P, 64], F32, tag="ones_f32")
            # ---- load xT first (gates first matmul), wv c0 interleaved ----
            xt = []
            wsl0 = []
            for kd in range(NT):
                t = bigp.tile([P, S], BF16, tag="big")
                nc.sync.dma_start(t[0:64, :], xT[kd * P : kd * P + 64, :])
                nc.sync.dma_start(t[64:P, :], xT[kd * P + 64 : (kd + 1) * P, :])
                xt.append(t)
                w0 = wtvp.tile([P, 512], BF16, tag="wtv", name=f"wv0_{kd}")
                nc.sync.dma_start(w0[:], wvT[kd * P : (kd + 1) * P, 0:512])
                wsl0.append(w0)
            nc.sync.dma_start(cos_t[:], cosx[:])
            nc.sync.dma_start(sin_t[:], sinx[:])
            nc.sync.dma_start(mask_t[:], maskm[:])
            nc.sync.dma_start(sel2[:], sel2d[:])
            nc.vector.memset(ones_f32[:], 1.0)
            warm = cp.tile([1, 8], F32, tag="warm")
            nc.scalar.activation(warm[:], ones_f32[0:1, 0:8], ACF.Exp)

            # ---- v projection into natural [S, 16*65] layout (ones cols) ----
            v65 = []
            for m in range(NT):
                t = vp.tile([P, H, 65], BF16, tag="v65")
                nc.scalar.activation(
                    t[:, :, 64:65],
                    ones_f32[:, 0:H].rearrange("p (h o) -> p h o", o=1),
                    ACF.Copy,
                )
                v65.append(t)
            for c in range(2):
                if c == 0:
                    wsl = wsl0
                else:
                    wsl = []
                    for kd in range(NT):
                        w = wtvp.tile([P, 512], BF16, tag="wtv")
                        nc.sync.dma_start(
                            w[:], wvT[kd * P : (kd + 1) * P, 512:1024]
                        )
                        wsl.append(w)
                for m in range(NT):
                    ps = psA.tile([P, 512], F32, tag="psA", name=f"psv{c}_{m}")
                    for kd in range(NT):
                        nc.tensor.matmul(
                            ps[:],
                            xt[kd][:, m * P : (m + 1) * P],
                            wsl[kd][:],
                            start=(kd == 0),
                            stop=(kd == NT - 1),
                        )
                    nc.scalar.activation(
                        v65[m][:, c * 8 : (c + 1) * 8, 0:64],
                        ps[:].rearrange("p (h d) -> p h d", d=64),
                        ACF.Copy,
                    )

            # ---- attention-out tiles ----
            ao = []
            for pt in range(NT):
                ao.append(aop.tile([P, S], BF16, tag="ao", name=f"ao{pt}"))

            def proj_one(w_dram, pt, kind):
                wt = wtp.tile([P, NT, P], BF16, tag="wt", name=f"wt{kind}{pt}")
                nc.sync.dma_start(
                    wt[:],
                    w_dram[:, pt * P : (pt + 1) * P].rearrange(
                        "(k p) i -> p k i", p=P
                    ),
                )
                plain = tmpp.tile([P, S], BF16, tag="plain", name=f"pl{kind}{pt}")
                for c in range(2):
                    ps = psA.tile([P, 512], F32, tag="psA", name=f"psp{kind}{pt}{c}")
                    for kd in range(NT):
                        nc.tensor.matmul(
                            ps[:],
                            wt[:, kd, :],
                            xt[kd][:, c * 512 : (c + 1) * 512],
                            start=(kd == 0),
                            stop=(kd == NT - 1),
                        )
                    nc.vector.tensor_copy(plain[:, c * 512 : (c + 1) * 512], ps[:])
                sw = tmpp.tile([P, S], BF16, tag="sw", name=f"sw{kind}{pt}")
                for blk in range(4):
                    srcp = (blk ^ 1) * 32
                    nc.sync.dma_start(
                        sw[blk * 32 : blk * 32 + 32, :],
                        plain[srcp : srcp + 32, :],
                    )
                rot = rotp.tile([P, S], BF16, tag="rot", name=f"rot{kind}{pt}")
                nc.vector.tensor_mul(rot[:], plain[:], cos_t[:])
                nc.vector.tensor_mul(sw[:], sw[:], sin_t[:])
                nc.vector.tensor_add(rot[:], rot[:], sw[:])
                return rot

            def normalize(pt):
                # ao[pt] *= 1/Z via rank-2 partition broadcast
                zpair = cp.tile([2, S], BF16, tag="zpair", name=f"zp{pt}", bufs=2)
                nc.gpsimd.dma_start(zpair[0:1, :], zpf[(pt, 0)][:])
                nc.gpsimd.dma_start(zpair[1:2, :], zpf[(pt, 1)][:])
                zb = psS.tile([P, S], F32, tag="psS", name=f"zb{pt}")
                for c in range(2):
                    nc.tensor.matmul(
                        zb[:, c * 512 : (c + 1) * 512],
                        sel2[:],
                        zpair[:, c * 512 : (c + 1) * 512],
                        start=True,
                        stop=True,
                    )
                for c in range(2):
                    nc.vector.tensor_mul(
                        ao[pt][:, c * 512 : (c + 1) * 512],
                        ao[pt][:, c * 512 : (c + 1) * 512],
                        zb[:, c * 512 : (c + 1) * 512],
                    )

            rots = {}
            rots[0] = (proj_one(wqT, 0, "q"), proj_one(wkT, 0, "k"))
            for pt in range(NT):
                if pt + 1 < NT:
                    rots[pt + 1] = (
                        proj_one(wqT, pt + 1, "q"),
                        proj_one(wkT, pt + 1, "k"),
                    )
                qrot, krot = rots.pop(pt)
                for half in range(2):
                    h = 2 * pt + half
                    hb = half * 64
                    oaccA = psO.tile([65, 512], F32, tag="psO", name=f"oaA{h}")
                    oaccB = psO.tile([65, 512], F32, tag="psO", name=f"oaB{h}")
                    # kt groups: kt<4 alone; (4,5) and (6,7) share a psS
                    # tile + exp. group = [(kt, tile_col_offset)].
                    for group in ([(0,)], [(1,)], [(2,)], [(3,)],
                                  [(4,), (5,)], [(6,), (7,)]):
                        kts = [g[0] for g in group]
                        g0 = kts[0]
                        sps = psS.tile([P, S], F32, tag="psS", name=f"s{h}_{g0}")
                        offs = {}
                        col = 0
                        for kt in kts:
                            qlo = kt * P
                            w = S - qlo
                            # place kt's [qlo, S) region at tile col `col`
                            # (chunks may not cross a 512B-bank boundary)
                            offs[kt] = col
                            a = qlo
                            while a < S:
                                b = min(S, a + 512 - ((col + a - qlo) % 512))
                                nc.tensor.matmul(
                                    sps[:, col + a - qlo : col + b - qlo],
                                    krot[hb : hb + 64, qlo : qlo + P],
                                    qrot[hb : hb + 64, a:b],
                                    start=True,
                                    stop=True,
                                )
                                a = b
                            col += w
                        et = expp.tile([P, S], BF16, tag="ex", name=f"e{h}_{g0}")
                        nc.scalar.activation(
                            et[:, 0:col], sps[:, 0:col], ACF.Exp, scale=0.125
                        )
                        for kt in kts:
                            qlo = kt * P
                            o = offs[kt]
                            nc.vector.tensor_mul(
                                et[:, o : o + P], et[:, o : o + P], mask_t[:]
                            )
                            avc = []
                            if qlo < 512:
                                avc.append((qlo, 512))
                            avc.append((max(512, qlo), S))
                            for (a, b) in avc:
                                tgt = (oaccA[:, a:b] if a < 512
                                       else oaccB[:, a - 512 : b - 512])
                                nc.tensor.matmul(
                                    tgt,
                                    v65[kt][:, h, :],
                                    et[:, o + a - qlo : o + b - qlo],
                                    start=(kt == 0),
                                    stop=(kt == NT - 1 if a >= 512 else kt == 3),
                                )
                    stage = stp.tile([65, S], BF16, tag="st", name=f"st{h}")
                    nc.vector.tensor_copy(stage[:, 0:512], oaccA[:])
                    nc.vector.tensor_copy(stage[:, 512:S], oaccB[:])
                    nc.sync.dma_start(ao[pt][hb : hb + 64, :], stage[0:64, :])
                    zh = cp.tile([1, S], F32, tag="zh", name=f"zh{h}", bufs=4)
                    nc.gpsimd.dma_start(zh[:], stage[64:65, :])
                    nc.vector.reciprocal(zh[:], zh[:])
                    zpf[(pt, half)] = zh
                if pt > 0:
                    normalize(pt - 1)
            normalize(NT - 1)

            # ---- final projection out[s, j] ----
            for c in range(2):
                wsl = []
                for kd in range(NT):
                    w = wtvp.tile([P, 512], BF16, tag="wtv")
                    nc.sync.dma_start(
                        w[:], woT[kd * P : (kd + 1) * P, c * 512 : (c + 1) * 512]
                    )
                    wsl.append(w)
                for m in range(NT):
                    ps = psA.tile([P, 512], F32, tag="psA", name=f"psf{c}_{m}")
                    for kd in range(NT):
                        nc.tensor.matmul(
                            ps[:],
                            ao[kd][:, m * P : (m + 1) * P],
                            wsl[kd][:],
                            start=(kd == 0),
                            stop=(kd == NT - 1),
                        )
                    ot = obp.tile([P, 512], F32, tag="ob")
                    nc.vector.tensor_copy(ot[:], ps[:])
                    nc.sync.dma_start(
                        out[m * P : (m + 1) * P, c * 512 : (c + 1) * 512], ot[:]
                    )

    nc.compile()
    return nc


_NC = None


def _host_prep(x, wq, wk, wv, wo, freqs_cos, freqs_sin):
    """Per-core input maps (host-side shuffles are free)."""
    # de-interleave permutation within each head: (2m, 2m+1) -> (m, m+32)
    perm = np.concatenate(
        [h * HD + np.concatenate([np.arange(0, HD, 2), np.arange(1, HD, 2)])
         for h in range(H)]
    )
    import ml_dtypes
    bf16 = ml_dtypes.bfloat16
    wqT = np.ascontiguousarray(wq[perm].T).astype(bf16)
    wkT = np.ascontiguousarray(wk[perm].T).astype(bf16)
    wvT = np.ascontiguousarray(wv.T).astype(bf16)
    woT = np.ascontiguousarray(wo.T).astype(bf16)
    cT = np.ascontiguousarray(freqs_cos.T, dtype=np.float32)  # [32, S]
    sT = np.ascontiguousarray(freqs_sin.T, dtype=np.float32)
    cosx = np.tile(cT, (4, 1)).astype(bf16)                    # [128, S]
    sinx = np.concatenate([-sT, sT, -sT, sT], axis=0).astype(bf16)
    kq = np.arange(P)
    maskm = (
        (kq[None, :] // BLK >= kq[:, None] // BLK).astype(bf16)
    )  # [k, q] multiplicative
    sel2 = np.zeros((2, P), dtype=bf16)
    sel2[0, 0:64] = 1.0
    sel2[1, 64:128] = 1.0
    shared = dict(wqT=wqT, wkT=wkT, wvT=wvT, woT=woT,
                  cosx=cosx, sinx=sinx, maskm=maskm, sel2=sel2)
    in_maps = []
    for b in range(N_CORES):
        m = dict(shared)
        m["xT"] = np.ascontiguousarray(x[b].T).astype(bf16)
        in_maps.append(m)
    return in_maps


def _run(inputs, trace=False):
    global _NC
    if _NC is None:
        _NC = _build()
    in_maps = _host_prep(**inputs)
    res = run_bass_kernel_spmd(
        _NC, in_maps, core_ids=list(range(N_CORES)), trace=trace
    )
    out = np.stack([res.results[i]["out"] for i in range(N_CORES)], axis=0)
    return out.astype(np.float32), res


def kernel(**inputs):
    inputs = {k: np.asarray(v) for k, v in inputs.items()}
    out, _ = _run(inputs, trace=False)
    return out


# revision 44
# speedup vs baseline: 1.1878x; 1.1878x over previous
"""Block-causal attention (B=8, S=1024, D=1024, H=16, hd=64) on 8 TRN2 cores.

Sharding: data-parallel over batch — core b computes batch b end-to-end,
weights replicated, no collectives.

Per-core layout strategy (all host-side prep is free):
  - host passes x[b].T           -> xT   [D, S]
  - host passes de-interleaved   -> wqT, wkT  [D, D]  (RoPE pairs (2m,2m+1)
    permuted to (m, m+32) within each head's 64 rows, then transposed)
  - host passes wv.T, wo.T       -> wvT, woT  [D, D]
  - qT,kT computed in [D, S] layout (stationary = weight tile)
  - v computed in natural [S, D] layout (stationary = xT tile), stored with a
    ones-column per head (65 cols) so the attn@v matmul also produces the
    softmax normalizer Z as psum row 64
  - scores computed transposed sT[k, q] per (head, k-tile); softmax over the
    partition dim k is folded into the v-matmul via the ones column
  - final out[s, j] computed naturally (stationary = attn-out tile), divided
    attn-out by Z beforehand via partition-broadcast multiply
"""

import sys

sys.path.insert(0, "/opt/trn_rl_repo")

import numpy as np

import concourse.bass as bass  # noqa: F401
import concourse.mybir as mybir
import concourse.tile as tile
from concourse import bacc
from concourse.bass_utils import run_bass_kernel_spmd

B, S, D, H, HD = 8, 1024, 1024, 16, 64
P = 128          # partitions / tile
NT = D // P      # 8 tiles along D or S
BLK = 8          # mask block size
N_CORES = 8
F32 = mybir.dt.float32

BF16 = mybir.dt.bfloat16


def _build():
    nc = bacc.Bacc(
        "TRN2", target_bir_lowering=False, debug=False, num_devices=N_CORES
    )
    xT = nc.dram_tensor("xT", [D, S], BF16, kind="ExternalInput").ap()
    wqT = nc.dram_tensor("wqT", [D, D], BF16, kind="ExternalInput").ap()
    wkT = nc.dram_tensor("wkT", [D, D], BF16, kind="ExternalInput").ap()
    wvT = nc.dram_tensor("wvT", [D, D], BF16, kind="ExternalInput").ap()
    woT = nc.dram_tensor("woT", [D, D], BF16, kind="ExternalInput").ap()
    cosx = nc.dram_tensor("cosx", [P, S], BF16, kind="ExternalInput").ap()
    sinx = nc.dram_tensor("sinx", [P, S], BF16, kind="ExternalInput").ap()
    maskm = nc.dram_tensor("maskm", [P, P], BF16, kind="ExternalInput").ap()
    sel2d = nc.dram_tensor("sel2", [2, P], BF16, kind="ExternalInput").ap()
    out = nc.dram_tensor("out", [S, D], F32, kind="ExternalOutput").ap()

    ACF = mybir.ActivationFunctionType

    with tile.TileContext(nc) as tc:
        with (
            tc.tile_pool(name="big", bufs=8) as bigp,      # xT tiles (bf16)
            tc.tile_pool(name="aop", bufs=8) as aop,       # attn-out tiles
            tc.tile_pool(name="rot", bufs=10) as rotp,      # qT_rot + kT_rot stream
            tc.tile_pool(name="v65", bufs=8) as vp,        # v with ones cols
            tc.tile_pool(name="wt", bufs=4) as wtp,        # q/k weight m-blocks
            tc.tile_pool(name="wtv", bufs=16) as wtvp,     # v/wo weight chunks
            tc.tile_pool(name="tmp", bufs=6) as tmpp,      # plain + swapped
            tc.tile_pool(name="ex", bufs=8) as expp,       # exp(scores) tiles
            tc.tile_pool(name="const", bufs=1) as cp,
            tc.tile_pool(name="ob", bufs=2) as obp,        # output staging
            tc.tile_pool(name="st", bufs=3) as stp,        # psum->sbuf stage
            tc.tile_pool(name="psA", bufs=2, space="PSUM") as psA,  # 2 banks
            tc.tile_pool(name="psS", bufs=2, space="PSUM") as psS,  # 4 banks
            tc.tile_pool(name="psO", bufs=2, space="PSUM") as psO,  # 2 banks
        ):
            # ---- constants ----
            cos_t = cp.tile([P, S], BF16, tag="cos")
            sin_t = cp.tile([P, S], BF16, tag="sin")
            mask_t = cp.tile([P, P], BF16, tag="mask")
            zpf = {}  # per-pair [2, S] f32 Z tiles
            sel2 = cp.tile([2, P], BF16, tag="sel2")
            ones_f32 = cp.tile([P, 64], F32, tag="ones_f32")
            # ---- load xT first (gates first matmul), wv c0 interleaved ----
            xt = []
            wsl0 = []
            for kd in range(NT):
                t = bigp.tile([P, S], BF16, tag="big")
                nc.sync.dma_start(t[0:64, :], xT[kd * P : kd * P + 64, :])
                nc.sync.dma_start(t[64:P, :], xT[kd * P + 64 : (kd + 1) * P, :])
                xt.append(t)
                w0 = wtvp.tile([P, 512], BF16, tag="wtv", name=f"wv0_{kd}")
                nc.sync.dma_start(w0[:], wvT[kd * P : (kd + 1) * P, 0:512])
                wsl0.append(w0)
            nc.sync.dma_start(cos_t[:], cosx[:])
            nc.sync.dma_start(sin_t[:], sinx[:])
            nc.sync.dma_start(mask_t[:], maskm[:])
            nc.sync.dma_start(sel2[:], sel2d[:])
            nc.vector.memset(ones_f32[:], 1.0)
            warm = cp.tile([1, 8], F32, tag="warm")
            nc.scalar.activation(warm[:], ones_f32[0:1, 0:8], ACF.Exp)

            # ---- v projection into natural [S, 16*65] layout (ones cols) ----
            v65 = []
            for m in range(NT):
                t = vp.tile([P, H, 65], BF16, tag="v65")
                nc.scalar.activation(
                    t[:, :, 64:65],
                    ones_f32[:, 0:H].rearrange("p (h o) -> p h o", o=1),
                    ACF.Copy,
                )
                v65.append(t)
            for c in range(2):
                if c == 0:
                    wsl = wsl0
                else:
                    wsl = []
                    for kd in range(NT):
                        w = wtvp.tile([P, 512], BF16, tag="wtv")
                        nc.sync.dma_start(
                            w[:], wvT[kd * P : (kd + 1) * P, 512:1024]
                        )
                        wsl.append(w)
                for m in range(NT):
                    ps = psA.tile([P, 512], F32, tag="psA", name=f"psv{c}_{m}")
                    for kd in range(NT):
                        nc.tensor.matmul(
                            ps[:],
                            xt[kd][:, m * P : (m + 1) * P],
                            wsl[kd][:],
                            start=(kd == 0),
                            stop=(kd == NT - 1),
                        )
                    nc.scalar.activation(
                        v65[m][:, c * 8 : (c + 1) * 8, 0:64],
                        ps[:].rearrange("p (h d) -> p h d", d=64),
                        ACF.Copy,
                    )

            # ---- attention-out tiles ----
            ao = []
            for pt in range(NT):
                ao.append(aop.tile([P, S], BF16, tag="ao", name=f"ao{pt}"))

            def proj_one(w_dram, pt, kind):
                wt = wtp.tile([P, NT, P], BF16, tag="wt", name=f"wt{kind}{pt}")
                nc.sync.dma_start(
                    wt[:],
                    w_dram[:, pt * P : (pt + 1) * P].rearrange(
                        "(k p) i -> p k i", p=P
                    ),
                )
                plain = tmpp.tile([P, S], BF16, tag="plain", name=f"pl{kind}{pt}")
                for c in range(2):
                    ps = psA.tile([P, 512], F32, tag="psA", name=f"psp{kind}{pt}{c}")
                    for kd in range(NT):
                        nc.tensor.matmul(
                            ps[:],
                            wt[:, kd, :],
                            xt[kd][:, c * 512 : (c + 1) * 512],
                            start=(kd == 0),
                            stop=(kd == NT - 1),
                        )
                    nc.vector.tensor_copy(plain[:, c * 512 : (c + 1) * 512], ps[:])
                sw = tmpp.tile([P, S], BF16, tag="sw", name=f"sw{kind}{pt}")
                for blk in range(4):
                    srcp = (blk ^ 1) * 32
                    nc.sync.dma_start(
                        sw[blk * 32 : blk * 32 + 32, :],
                        plain[srcp : srcp + 32, :],
                    )
                rot = rotp.tile([P, S], BF16, tag="rot", name=f"rot{kind}{pt}")
                nc.vector.tensor_mul(rot[:], plain[:], cos_t[:])
                nc.vector.tensor_mul(sw[:], sw[:], sin_t[:])
                nc.vector.tensor_add(rot[:], rot[:], sw[:])
                return rot

            def normalize(pt):
                # ao[pt] *= 1/Z via rank-2 partition broadcast
                zpair = cp.tile([2, S], BF16, tag="zpair", name=f"zp{pt}", bufs=2)
                nc.gpsimd.dma_start(zpair[0:1, :], zpf[(pt, 0)][:])
                nc.gpsimd.dma_start(zpair[1:2, :], zpf[(pt, 1)][:])
                zb = psS.tile([P, S], F32, tag="psS", name=f"zb{pt}")
                for c in range(2):
                    nc.tensor.matmul(
                        zb[:, c * 512 : (c + 1) * 512],
                        sel2[:],
                        zpair[:, c * 512 : (c + 1) * 512],
                        start=True,
                        stop=True,
                    )
                for c in range(2):
                    nc.vector.tensor_mul(
                        ao[pt][:, c * 512 : (c + 1) * 512],
                        ao[pt][:, c * 512 : (c + 1) * 512],
                        zb[:, c * 512 : (c + 1) * 512],
                    )

            rots = {}
            rots[0] = (proj_one(wqT, 0, "q"), proj_one(wkT, 0, "k"))
            for pt in range(NT):
                if pt + 1 < NT:
                    rots[pt + 1] = (
                        proj_one(wqT, pt + 1, "q"),
                        proj_one(wkT, pt + 1, "k"),
                    )
                qrot, krot = rots.pop(pt)
                for half in range(2):
                    h = 2 * pt + half
                    hb = half * 64
                    oaccA = psO.tile([65, 512], F32, tag="psO", name=f"oaA{h}")
                    oaccB = psO.tile([65, 512], F32, tag="psO", name=f"oaB{h}")
                    for kt in range(NT):
                        qlo = kt * P
                        w = S - qlo
                        sps = psS.tile([P, S], F32, tag="psS", name=f"s{h}_{kt}")
                        chunks = []
                        if qlo < 512:
                            chunks.append((qlo, 512))
                        chunks.append((max(512, qlo), S))
                        for (a, b) in chunks:
                            nc.tensor.matmul(
                                sps[:, a:b],
                                krot[hb : hb + 64, qlo : qlo + P],
                                qrot[hb : hb + 64, a:b],
                                start=True,
                                stop=True,
                            )
                        et = expp.tile([P, S], BF16, tag="ex", name=f"e{h}_{kt}")
                        nc.scalar.activation(
                            et[:, 0:w], sps[:, qlo:S], ACF.Exp, scale=0.125
                        )
                        nc.vector.tensor_mul(et[:, 0:P], et[:, 0:P], mask_t[:])
                        avc = []
                        if qlo < 512:
                            avc.append((qlo, 512))
                        avc.append((max(512, qlo), S))
                        for (a, b) in avc:
                            tgt = oaccA[:, a:b] if a < 512 else oaccB[:, a - 512 : b - 512]
                            nc.tensor.matmul(
                                tgt,
                                v65[kt][:, h, :],
                                et[:, a - qlo : b - qlo],
                                start=(kt == 0),
                                stop=(kt == NT - 1 if a >= 512 else kt == 3),
                            )
                    stage = stp.tile([65, S], BF16, tag="st", name=f"st{h}")
                    nc.vector.tensor_copy(stage[:, 0:512], oaccA[:])
                    nc.vector.tensor_copy(stage[:, 512:S], oaccB[:])
                    nc.sync.dma_start(ao[pt][hb : hb + 64, :], stage[0:64, :])
                    zh = cp.tile([1, S], F32, tag="zh", name=f"zh{h}", bufs=4)
                    nc.gpsimd.dma_start(zh[:], stage[64:65, :])
                    nc.vector.reciprocal(zh[:], zh[:])
                    zpf[(pt, half)] = zh
                if pt > 0:
                    normalize(pt - 1)
            normalize(NT - 1)

            # ---- final projection out[s, j] ----
            for c in range(2):
                wsl = []
                for kd in range(NT):
                    w = wtvp.tile([P, 512], BF16, tag="wtv")
                    nc.sync.dma_start(
                        w[:], woT[kd * P : (kd + 1) * P, c * 512 : (c + 1) * 512]
                    )
                    wsl.append(w)
                for m in range(NT):
                    ps = psA.tile([P, 512], F32, tag="psA", name=f"psf{c}_{m}")
                    for kd in range(NT):
                        nc.tensor.matmul(
                            ps[:],
                            ao[kd][:, m * P : (m + 1) * P],
                            wsl[kd][:],
                            start=(kd == 0),
                            stop=(kd == NT - 1),
                        )
                    ot = obp.tile([P, 512], F32, tag="ob")
                    nc.vector.tensor_copy(ot[:], ps[:])
                    nc.sync.dma_start(
                        out[m * P : (m + 1) * P, c * 512 : (c + 1) * 512], ot[:]
                    )

    nc.compile()
    return nc


_NC = None


def _host_prep(x, wq, wk, wv, wo, freqs_cos, freqs_sin):
    """Per-core input maps (host-side shuffles are free)."""
    # de-interleave permutation within each head: (2m, 2m+1) -> (m, m+32)
    perm = np.concatenate(
        [h * HD + np.concatenate([np.arange(0, HD, 2), np.arange(1, HD, 2)])
         for h in range(H)]
    )
    import ml_dtypes
    bf16 = ml_dtypes.bfloat16
    wqT = np.ascontiguousarray(wq[perm].T).astype(bf16)
    wkT = np.ascontiguousarray(wk[perm].T).astype(bf16)
    wvT = np.ascontiguousarray(wv.T).astype(bf16)
    woT = np.ascontiguousarray(wo.T).astype(bf16)
    cT = np.ascontiguousarray(freqs_cos.T, dtype=np.float32)  # [32, S]
    sT = np.ascontiguousarray(freqs_sin.T, dtype=np.float32)
    cosx = np.tile(cT, (4, 1)).astype(bf16)                    # [128, S]
    sinx = np.concatenate([-sT, sT, -sT, sT], axis=0).astype(bf16)
    kq = np.arange(P)
    maskm = (
        (kq[None, :] // BLK >= kq[:, None] // BLK).astype(bf16)
    )  # [k, q] multiplicative
    sel2 = np.zeros((2, P), dtype=bf16)
    sel2[0, 0:64] = 1.0
    sel2[1, 64:128] = 1.0
    shared = dict(wqT=wqT, wkT=wkT, wvT=wvT, woT=woT,
                  cosx=cosx, sinx=sinx, maskm=maskm, sel2=sel2)
    in_maps = []
    for b in range(N_CORES):
        m = dict(shared)
        m["xT"] = np.ascontiguousarray(x[b].T).astype(bf16)
        in_maps.append(m)
    return in_maps


def _run(inputs, trace=False):
    global _NC
    if _NC is None:
        _NC = _build()
    in_maps = _host_prep(**inputs)
    res = run_bass_kernel_spmd(
        _NC, in_maps, core_ids=list(range(N_CORES)), trace=trace
    )
    out = np.stack([res.results[i]["out"] for i in range(N_CORES)], axis=0)
    return out.astype(np.float32), res


def kernel(**inputs):
    inputs = {k: np.asarray(v) for k, v in inputs.items()}
    out, _ = _run(inputs, trace=False)
    return out
